# revision 1
# baseline (speedup 1.0000x reference)
"""
W8A8 quantized linear (dynamic per-token int8 activation quant + int8 weight,
fp32 dequant) on 8 Trainium2 NeuronCores — v2.

Changes vs v1 baseline (955 us):
  * weight_scales folded into the weight on host (bf16(w*ws), rel err ~1.7e-3
    vs 2e-2 budget) -> PSUM evacuation is a single ACT copy-with-scale, no DVE
    tensor_tensor and no wsb tile.
  * weight pre-swizzled on host to [NS, 128, KT, NSL] so each n-slice loads as
    4 fully-contiguous 1 MiB DMA chunks (finer matmul deps + max DMA bw).
  * PE warm-up: dummy back-to-back matmuls issued at kernel start so the HAM
    clock-gate un-throttles (1.2 -> 2.4 GHz) while the quant prefix runs.
  * quant phase pipelined per m-tile into phase C via subtile deps.

Sharding: data-parallel over tokens (8192 tokens -> 1024/core); weight
replicated.
"""

import numpy as np
import ml_dtypes
from contextlib import ExitStack

import concourse.bass as bass
import concourse.mybir as mybir
import concourse.tile as tile
from concourse import bacc

QMAX = 127.0
MAGIC = 1.5 * 2**23  # fp32 round-to-nearest-even trick for |v| < 2^22

F16 = mybir.dt.float16
BF16 = mybir.dt.bfloat16
F32 = mybir.dt.float32


def build_nc(M=1024, K=4096, N=4096, NSL=512, QCH=1024, WCH=8,
             warm_mms=28, reps=1, do_quant=True, do_mm=True):
    """One-core program; run SPMD on 8 cores with different token shards."""
    nc = bacc.Bacc()
    MT, KT, NS = M // 128, K // 128, N // NSL
    NWC = KT // WCH  # weight DMA chunks per n-slice

    x = nc.declare_dram_parameter("x", [M, K], F16, isOutput=False)
    w5 = nc.declare_dram_parameter("w5", [NS, 128, KT, NSL], BF16, isOutput=False)
    y = nc.declare_dram_parameter("y", [M, N], F16, isOutput=True)

    with tile.TileContext(nc) as tc, ExitStack() as ctx:
      pers = ctx.enter_context(tc.tile_pool(name="pers", bufs=1))
      qpool = ctx.enter_context(tc.tile_pool(name="qt", bufs=1))
      xpool = ctx.enter_context(tc.tile_pool(name="xa", bufs=3))
      tpool = ctx.enter_context(tc.tile_pool(name="tmpq", bufs=2))
      qnat = ctx.enter_context(tc.tile_pool(name="qnat", bufs=2))
      wpool = ctx.enter_context(tc.tile_pool(name="wt", bufs=2))
      psum = ctx.enter_context(tc.tile_pool(name="psum", bufs=7, space="PSUM"))
      opool = ctx.enter_context(tc.tile_pool(name="out", bufs=3))

      for rep in range(reps):
        if rep > 0:
            tc.strict_bb_all_engine_barrier()

        # ---- PE warm-up: back-to-back dummy matmuls (WAW-serialized) so the
        # HAM activity monitor lifts the 1.2 GHz clock gate during the quant
        # prefix and phase C starts at 2.4 GHz.
        if warm_mms:
            dwarm = pers.tile([128, 512], BF16)
            nc.vector.memset(dwarm[:], 1.0)
            dps = psum.tile([128, 512], F32, tag="pt")
            for _ in range(warm_mms):
                nc.tensor.matmul(dps[:], dwarm[:, 0:128], dwarm[:],
                                 start=True, stop=True)

        # SP HWDGE ring: interleave x tiles with the first weight slice's
        # chunks so neither the quant chain nor the first matmuls stall.
        xa_tiles = [None] * MT
        wts0 = [None] * NWC
        def _emit_x(mt):
            xa = xpool.tile([128, K], F16, tag="xa")
            nc.sync.dma_start(xa[:], x[mt * 128 : (mt + 1) * 128, :])
            xa_tiles[mt] = xa
        def _emit_w0(c):
            wt = wpool.tile([128, WCH, NSL], BF16, tag=f"wt{c}")
            nc.sync.dma_start(wt[:], w5[0, :, c * WCH : (c + 1) * WCH, :])
            wts0[c] = wt
        if do_quant and do_mm:
            _emit_x(0); _emit_w0(0); _emit_w0(1); _emit_x(1); _emit_w0(2)
            _emit_x(2); _emit_w0(3)
            for mt in range(3, MT):
                _emit_x(mt)
        else:
            if do_quant:
                for mt in range(MT):
                    _emit_x(mt)
            if do_mm:
                for c in range(NWC):
                    _emit_w0(c)

        am = pers.tile([128, MT], F32)
        scales = pers.tile([128, MT], F32)
        invs = pers.tile([128, MT], F32)
        # m-tile-major so each m-tile's [KT,128] block is contiguous: one
        # xbar transpose per m-tile writes qT[p, mt, kt, m] = q[mt*128+m, kt*128+p]
        qT = qpool.tile([128, MT, KT, 128], BF16)

        if not do_quant:
            nc.vector.memset(scales[:], 1.0)
            if do_mm:
                nc.vector.memset(qT[:], 1.0)
        # ---- phase A/B: per m-tile absmax, scales, quantize, transpose ----
        for mt in range(MT if do_quant else 0):
            xa = xa_tiles[mt]
            nc.vector.tensor_reduce(
                am[:, mt : mt + 1],
                xa[:],
                axis=mybir.AxisListType.X,
                op=mybir.AluOpType.max,
                apply_absolute_value=True,
            )
            # scale = max(absmax/127, 1e-8); inv = 1/scale
            nc.vector.tensor_scalar(
                scales[:, mt : mt + 1],
                am[:, mt : mt + 1],
                1.0 / QMAX,
                1e-8,
                mybir.AluOpType.mult,
                mybir.AluOpType.max,
            )
            nc.vector.reciprocal(invs[:, mt : mt + 1], scales[:, mt : mt + 1])

            qn = qnat.tile([128, K], BF16, tag="qn")
            for kc in range(K // QCH):
                sl = slice(kc * QCH, (kc + 1) * QCH)
                tmpq = tpool.tile([128, QCH], F32, tag="tmpq")
                # tmpq = x*inv + MAGIC  (fp32; rounds to integer at +MAGIC)
                nc.vector.tensor_scalar(
                    tmpq[:],
                    xa[:, sl],
                    invs[:, mt : mt + 1],
                    MAGIC,
                    mybir.AluOpType.mult,
                    mybir.AluOpType.add,
                )
                # qn = tmpq - MAGIC  (exact; integer-valued, exact in bf16)
                nc.scalar.activation(
                    qn[:, sl],
                    tmpq[:],
                    mybir.ActivationFunctionType.Copy,
                    bias=-MAGIC,
                )

            # one xbar transpose for the whole m-tile: [128m, 4096k] ->
            # [128k-part, KT, 128m] (contiguous dst block)
            nc.scalar.dma_start_transpose(qT[:, mt], qn[:, :])

        if not do_mm:
            ot0 = opool.tile([128, NSL], F16, tag="ot")
            nc.vector.memset(ot0[:], 0.0)
            nc.scalar.dma_start(y[0:128, 0:NSL], ot0[:])
        # ---- phase C: matmul + dequant (scale only; wscales folded in w5) ----
        for ns in range(NS if do_mm else 0):
            nsl = slice(ns * NSL, (ns + 1) * NSL)
            if ns == 0:
                wts = wts0
            else:
                wts = []
                for c in range(NWC):
                    wt = wpool.tile([128, WCH, NSL], BF16, tag=f"wt{c}")
                    nc.sync.dma_start(wt[:], w5[ns, :, c * WCH : (c + 1) * WCH, :])
                    wts.append(wt)
            for mt in range(MT):
                pt = psum.tile([128, NSL], F32, tag="pt")
                for kt in range(KT):
                    nc.tensor.matmul(
                        pt[:],
                        qT[:, mt, kt, :],
                        wts[kt // WCH][:, kt % WCH, :],
                        start=(kt == 0),
                        stop=(kt == KT - 1),
                    )
                # evacuate on ACT (measured ~110us faster than DVE evac:
                # DVE PSUM reads interfere with concurrent weight DMA)
                ot = opool.tile([128, NSL], F16, tag="ot")
                nc.scalar.activation(
                    ot[:],
                    pt[:],
                    mybir.ActivationFunctionType.Copy,
                    bias=0.0,
                    scale=scales[:, mt : mt + 1],
                )
                nc.scalar.dma_start(y[mt * 128 : (mt + 1) * 128, nsl], ot[:])

    nc.finalize()
    return nc


def prep_inputs(x, weight, weight_scales, n_cores=8, NSL=512):
    """Host-side shard/layout prep. Returns (in_maps, out_assembler)."""
    B, S, D_in = x.shape
    D_out = weight.shape[0]
    M_total = B * S
    Mc = M_total // n_cores
    NS, KT = D_out // NSL, D_in // 128

    xf = np.ascontiguousarray(np.asarray(x).reshape(M_total, D_in))
    wf = np.asarray(weight).astype(np.float32) \
        * np.asarray(weight_scales).astype(np.float32)[:, None]   # [N, K]
    # W5[ns, p, kt, j] = wf[ns*NSL+j, kt*128+p]
    W5 = np.ascontiguousarray(
        wf.reshape(NS, NSL, KT, 128).transpose(0, 3, 2, 1)
    ).astype(ml_dtypes.bfloat16)

    in_maps = [
        {"x": xf[c * Mc : (c + 1) * Mc], "w5": W5}
        for c in range(n_cores)
    ]

    def assemble(results):
        return np.concatenate(
            [np.asarray(results[c]["y"]) for c in range(n_cores)], axis=0
        ).reshape(B, S, D_out).astype(np.float16)

    return in_maps, assemble


def kernel(x, weight, weight_scales):
    from concourse.bass_utils import run_bass_kernel_spmd

    n_cores = 8
    B, S, D_in = x.shape
    D_out = weight.shape[0]
    Mc = (B * S) // n_cores

    nc = build_nc(M=Mc, K=D_in, N=D_out)
    in_maps, assemble = prep_inputs(x, weight, weight_scales, n_cores)
    res = run_bass_kernel_spmd(nc, in_maps, list(range(n_cores)))
    return assemble(res.results)


if __name__ == "__main__":
    np.random.seed(0)
    x = np.random.randn(4, 2048, 4096).astype(np.float16)
    w = np.random.randint(-127, 127, (4096, 4096)).astype(np.int8)
    ws = (np.random.rand(4096).astype(np.float32) * 0.01 + 1e-4).astype(np.float16)
    y = kernel(x, w, ws)
    print(y.shape, y.dtype)



# revision 2
# speedup vs baseline: 1.0232x; 1.0232x over previous
"""
W8A8 quantized linear (dynamic per-token int8 activation quant + int8 weight,
fp32 dequant) on 8 Trainium2 NeuronCores — v3.

Changes vs v2 (525 us measured):
  * quantize = ONE DVE tensor_scalar pass per chunk: q1536 = x*inv + 1536 with
    fp16 output — the fp16 cast itself rounds-to-nearest-even (magic-number
    trick: 1536 = 1.5*2^10, so 1536+q for |q|<=127 has ulp 1 in fp16).
    Runs in DVE 4x mode (16-bit in/out, SBUF, stride 1): ~0.6 us per 2048-col
    chunk vs the old fp32 two-op chain. The -1536 removal runs on the Scalar
    engine (ACT Copy with bias), which is otherwise idle during the prefix.
  * per-m-tile quant pipeline rate drops ~12.2 us -> ~6.0 us, below the
    6.9 us/m-tile matmul consumption rate, so phase C no longer starves
    (v2 lost ~85 us of PE idle + HAM re-throttling in the first 105 us).
  * x DMA split into half-tiles so the absmax reduce overlaps the load;
    all quant ops chunked at 2048 cols, transposes at half-m-tile grain.
  * weight fold w*ws stored in fp16 (max |w*ws| ~ 1.28, all normal): rel
    fold error 2^-11 vs bf16's 2^-8.
  * weight slice 1 prefetched up front so the transposes on the sync queue
    cannot delay it.

Sharding: data-parallel over tokens (8192 tokens -> 1024/core); weight
replicated.
"""

import numpy as np
import ml_dtypes
from contextlib import ExitStack

import concourse.bass as bass
import concourse.mybir as mybir
import concourse.tile as tile
from concourse import bacc

QMAX = 127.0
MAGIC = 1536.0  # 1.5 * 2^10: fp16 round-to-nearest-even trick for |v| <= 255

F16 = mybir.dt.float16
F32 = mybir.dt.float32


def build_nc(M=1024, K=4096, N=4096, NSL=512, WCH=8, warm_mms=28):
    """One-core program; run SPMD on 8 cores with different token shards."""
    nc = bacc.Bacc()
    MT, KT, NS = M // 128, K // 128, N // NSL
    NWC = KT // WCH      # weight DMA chunks per n-slice
    HK = K // 2          # half m-tile columns (2048)
    HKT = KT // 2        # k-tiles per half (16)

    x = nc.declare_dram_parameter("x", [M, K], F16, isOutput=False)
    w5 = nc.declare_dram_parameter("w5", [NS, 128, KT, NSL], F16, isOutput=False)
    y = nc.declare_dram_parameter("y", [M, N], F16, isOutput=True)

    with tile.TileContext(nc) as tc, ExitStack() as ctx:
      pers = ctx.enter_context(tc.tile_pool(name="pers", bufs=1))
      qpool = ctx.enter_context(tc.tile_pool(name="qt", bufs=1))
      xpool = ctx.enter_context(tc.tile_pool(name="xa", bufs=3))
      tpool = ctx.enter_context(tc.tile_pool(name="q1536", bufs=3))
      qnat = ctx.enter_context(tc.tile_pool(name="qn", bufs=3))
      wpool = ctx.enter_context(tc.tile_pool(name="wt", bufs=2))
      psum = ctx.enter_context(tc.tile_pool(name="psum", bufs=7, space="PSUM"))
      opool = ctx.enter_context(tc.tile_pool(name="out", bufs=3))

      # ---- PE warm-up: back-to-back dummy matmuls (WAW-serialized) so the
      # HAM activity monitor lifts the 1.2 GHz clock gate while the first
      # x tiles load + quantize, and phase C starts at 2.4 GHz.
      if warm_mms:
          dwarm = pers.tile([128, 512], F16)
          nc.vector.memset(dwarm[:], 1.0)
          dps = psum.tile([128, 512], F32, tag="pt")
          for _ in range(warm_mms):
              nc.tensor.matmul(dps[:], dwarm[:, 0:128], dwarm[:],
                               start=True, stop=True)

      # SP HWDGE ring: x half-tiles (so the reduce can start on the first
      # half while the second streams) interleaved with the first TWO weight
      # slices' chunks (slice 1 up front so the transposes that follow on
      # this queue cannot delay it).
      xa_tiles = [None] * MT
      wts01 = [[None] * NWC, [None] * NWC]
      def _emit_x(mt):
          xa = xpool.tile([128, K], F16, tag="xa")
          for h in range(2):
              nc.sync.dma_start(xa[:, h * HK:(h + 1) * HK],
                                x[mt * 128:(mt + 1) * 128, h * HK:(h + 1) * HK])
          xa_tiles[mt] = xa
      def _emit_w(ns, c):
          wt = wpool.tile([128, WCH, NSL], F16, tag=f"wt{c}")
          nc.sync.dma_start(wt[:], w5[ns, :, c * WCH:(c + 1) * WCH, :])
          wts01[ns][c] = wt
      _emit_x(0); _emit_w(0, 0); _emit_w(0, 1); _emit_x(1); _emit_w(0, 2)
      _emit_x(2); _emit_w(0, 3)
      for mt in range(3, MT):
          _emit_x(mt)
          if mt - 3 < NWC:
              _emit_w(1, mt - 3)

      am2 = pers.tile([128, MT, 2], F32)
      am = pers.tile([128, MT], F32)
      scales = pers.tile([128, MT], F32)
      invs = pers.tile([128, MT], F32)
      # m-tile-major so each m-tile's [KT,128] block is contiguous:
      # qT[p, mt, kt, m] = q[mt*128+m, kt*128+p]
      qT = qpool.tile([128, MT, KT, 128], F16)

      # ---- phase A/B: per m-tile absmax, scales, quantize, transpose ----
      for mt in range(MT):
          xa = xa_tiles[mt]
          for h in range(2):
              nc.vector.tensor_reduce(
                  am2[:, mt, h:h + 1],
                  xa[:, h * HK:(h + 1) * HK],
                  axis=mybir.AxisListType.X,
                  op=mybir.AluOpType.max,
                  apply_absolute_value=True,
              )
          nc.vector.tensor_reduce(
              am[:, mt:mt + 1],
              am2[:, mt, :],
              axis=mybir.AxisListType.X,
              op=mybir.AluOpType.max,
          )
          # scale = max(absmax/127, 1e-8); inv = 1/scale
          nc.vector.tensor_scalar(
              scales[:, mt:mt + 1],
              am[:, mt:mt + 1],
              1.0 / QMAX,
              1e-8,
              mybir.AluOpType.mult,
              mybir.AluOpType.max,
          )
          nc.vector.reciprocal(invs[:, mt:mt + 1], scales[:, mt:mt + 1])

          for h in range(2):
              sl = slice(h * HK, (h + 1) * HK)
              # q1536 = fp16(x*inv + 1536): the fp16 cast rounds to integer
              # (+1536); DVE 4x mode (16-bit, single-src, SBUF)
              t1536 = tpool.tile([128, HK], F16, tag="t1536")
              nc.vector.tensor_scalar(
                  t1536[:],
                  xa[:, sl],
                  invs[:, mt:mt + 1],
                  MAGIC,
                  mybir.AluOpType.mult,
                  mybir.AluOpType.add,
              )
              # qn = q1536 - 1536 (exact int in fp16), on the Scalar engine
              qn = qnat.tile([128, HK], F16, tag="qn")
              nc.scalar.activation(
                  qn[:],
                  t1536[:],
                  mybir.ActivationFunctionType.Copy,
                  bias=-MAGIC,
              )
              # xbar transpose of the half m-tile: [128m, 2048k] ->
              # [128k-part, HKT, 128m] (contiguous dst block)
              nc.sync.dma_start_transpose(
                  qT[:, mt, h * HKT:(h + 1) * HKT, :], qn[:, :])

      # ---- phase C: matmul + dequant (scale only; wscales folded in w5) ----
      for ns in range(NS):
          nsl = slice(ns * NSL, (ns + 1) * NSL)
          if ns < 2:
              wts = wts01[ns]
          else:
              wts = []
              for c in range(NWC):
                  wt = wpool.tile([128, WCH, NSL], F16, tag=f"wt{c}")
                  nc.sync.dma_start(wt[:], w5[ns, :, c * WCH:(c + 1) * WCH, :])
                  wts.append(wt)
          for mt in range(MT):
              pt = psum.tile([128, NSL], F32, tag="pt")
              for kt in range(KT):
                  nc.tensor.matmul(
                      pt[:],
                      qT[:, mt, kt, :],
                      wts[kt // WCH][:, kt % WCH, :],
                      start=(kt == 0),
                      stop=(kt == KT - 1),
                  )
              # evacuate on ACT (DVE PSUM reads interfere with weight DMA)
              ot = opool.tile([128, NSL], F16, tag="ot")
              nc.scalar.activation(
                  ot[:],
                  pt[:],
                  mybir.ActivationFunctionType.Copy,
                  bias=0.0,
                  scale=scales[:, mt:mt + 1],
              )
              nc.scalar.dma_start(y[mt * 128:(mt + 1) * 128, nsl], ot[:])

    nc.finalize()
    return nc


def prep_inputs(x, weight, weight_scales, n_cores=8, NSL=512):
    """Host-side shard/layout prep. Returns (in_maps, out_assembler)."""
    B, S, D_in = x.shape
    D_out = weight.shape[0]
    M_total = B * S
    Mc = M_total // n_cores
    NS, KT = D_out // NSL, D_in // 128

    xf = np.ascontiguousarray(np.asarray(x).reshape(M_total, D_in))
    wf = np.asarray(weight).astype(np.float32) \
        * np.asarray(weight_scales).astype(np.float32)[:, None]   # [N, K]
    # W5[ns, p, kt, j] = wf[ns*NSL+j, kt*128+p]
    W5 = np.ascontiguousarray(
        wf.reshape(NS, NSL, KT, 128).transpose(0, 3, 2, 1)
    ).astype(np.float16)

    in_maps = [
        {"x": xf[c * Mc:(c + 1) * Mc], "w5": W5}
        for c in range(n_cores)
    ]

    def assemble(results):
        return np.concatenate(
            [np.asarray(results[c]["y"]) for c in range(n_cores)], axis=0
        ).reshape(B, S, D_out).astype(np.float16)

    return in_maps, assemble


def kernel(x, weight, weight_scales):
    from concourse.bass_utils import run_bass_kernel_spmd

    n_cores = 8
    B, S, D_in = x.shape
    D_out = weight.shape[0]
    Mc = (B * S) // n_cores

    nc = build_nc(M=Mc, K=D_in, N=D_out)
    in_maps, assemble = prep_inputs(x, weight, weight_scales, n_cores)
    res = run_bass_kernel_spmd(nc, in_maps, list(range(n_cores)))
    return assemble(res.results)


if __name__ == "__main__":
    np.random.seed(0)
    x = np.random.randn(4, 2048, 4096).astype(np.float16)
    w = np.random.randint(-127, 127, (4096, 4096)).astype(np.int8)
    ws = (np.random.rand(4096).astype(np.float32) * 0.01 + 1e-4).astype(np.float16)
    y = kernel(x, w, ws)
    print(y.shape, y.dtype)
